# revision 29
# baseline (speedup 1.0000x reference)
"""Trainium2 Bass kernel for nn_Attention_p_2757369004155.

Reference math (per p in 0..4):
  x [256, 1728] -> qkv = W_qkv @ x -> 8 heads of dim 32, N=1728
  attn = softmax((q*scale)^T k), out = v @ attn^T, y = W_p @ out + b

Sharding: 8 cores = 4 p-branches x 2 query-halves. Each core is fully
self-contained (K/V computed for all n, Q for its half). The host permutes
each core's n axis so its query half is always columns [0, 864).

Engine budget per core (cost-model cycles at 2.4 GHz):
  - QKV + projection matmuls run as f32r (1 cycle/row at ap>=256, vs fp32's
    4): K/Q/V/proj ~ 8.8 us total on PE.
  - S^T (QK^T) runs per-head as single-pass f32r (contraction 32, no
    residual stack): 8 heads x 14 n-tiles x 864 m ~ 40 us. Per-head operand
    slices address k_sb/q_sb partitions directly, so no stack-building DMAs.
  - exp(S^T) is the largest elementwise load (11.9M elements). It is split
    across three engines: ACT computes real exp (PSUM->SBUF bf16), DVE and
    Pool compute a Schraudolph bit-trick exp (one tensor_scalar op:
    i16 = x*128/ln2 + 16252.5 viewed as bf16), which is accurate to ~2-3%
    per element and dilutes to <1e-3 after softmax averaging (validated in
    numpy against the reference; tolerance is 2e-2).
  - A@V runs in the O^T[m, c] orientation with bf16 operands (1 cycle/row):
    exp tiles as stationary, V^T (with a ones column for the softmax
    denominators) as moving: ~11 us on PE.
  - Softmax normalization is a per-partition scaled PSUM->SBUF copy into
    bf16 O^T tiles; the O^T -> O transpose uses the DMA xbar transpose
    (16x128 tiles, off the PE entirely); the final projection is bf16.
"""

import math

import numpy as np

import concourse.bass as bass
import concourse.tile as tile
from concourse import bacc, mybir
from concourse.bass import ds
from concourse.bass_utils import run_bass_kernel_spmd

F32 = mybir.dt.float32
F32R = mybir.dt.float32r
BF16 = mybir.dt.bfloat16
I16 = mybir.dt.int16
AF = mybir.ActivationFunctionType
ALU = mybir.AluOpType

N_CORES = 8
C = 256            # channels
NH = 8             # heads
HD = 32            # head dim
N = 1728           # sequence (12*12*12)
M = N // 2         # per-core query positions
MC = 432           # m chunk (PSUM bank = 512 fp32)
NT_SIZES = [128] * 13 + [64]          # n contraction tiles
MT_SIZES = [128] * 6 + [96]           # m tiles for the O^T matmuls
SCALE = HD ** -0.5

# Schraudolph exp in bf16-bit space: i16 = round(x * 128/ln2 + B).
# B = 127*128 - C + 0.5; C=4 balances the piecewise-linear 2^frac error.
EXP_A = 128.0 / math.log(2.0)
EXP_B = 127.0 * 128.0 - 4.0 + 0.5


def build_program():
    nc = bacc.Bacc(
        "TRN2",
        target_bir_lowering=False,
        debug=False,
        enable_asserts=False,
        num_devices=N_CORES,
    )

    xf_d = nc.dram_tensor("xf", [C, N], F32R, kind="ExternalInput").ap()
    wq_d = nc.dram_tensor("wqT", [C, C], F32R, kind="ExternalInput").ap()
    wk_d = nc.dram_tensor("wkT", [C, C], F32R, kind="ExternalInput").ap()
    wv_d = nc.dram_tensor("wvT", [C, C], F32R, kind="ExternalInput").ap()
    wp_d = nc.dram_tensor("wpT", [C, C], BF16, kind="ExternalInput").ap()
    b_d = nc.dram_tensor("bias", [C, 1], F32, kind="ExternalInput").ap()
    y_d = nc.dram_tensor("y", [C, M], F32, kind="ExternalOutput").ap()

    xf_r = xf_d.rearrange("(kt p) n -> p kt n", p=128)

    with tile.TileContext(nc) as tc:
        with (
            tc.tile_pool(name="persist", bufs=1) as sb,
            tc.tile_pool(name="rot", bufs=3) as rot,
        ):
            # ---- persistent SBUF tiles ----
            wk_sb = sb.tile([128, 2, 2, 128], F32R, tag="wk")
            wq_sb = sb.tile([128, 2, 2, 128], F32R, tag="wq")
            wv_sb = sb.tile([128, 2, C], F32R, tag="wv")
            wp_sb = sb.tile([128, 2, 2, 128], BF16, tag="wp")
            b_sb = sb.tile([128, 2, 1], F32, tag="b")
            xf_sb = sb.tile([128, 2, N], F32R, tag="xf")
            k_sb = sb.tile([128, 2, N], F32R, tag="k")
            q_sb = sb.tile([128, 2, M], F32R, tag="q")
            # PE operands must sit at partition base 0/32/64: heads 0-2 of
            # each group are addressed directly in k_sb/q_sb; head 3 (base
            # 96) is DMA-relocated to these base-0 tiles.
            k3_sb = sb.tile([32, 2, N], F32R, tag="k3")
            q3_sb = sb.tile([32, 2, M], F32R, tag="q3")
            # V^T tiles with a ones column per head ([n, nt, head, 33]);
            # col 33 is alignment padding.
            vt_sb = sb.tile([128, 14, NH, 34], BF16, tag="vt")
            on_t_sb = sb.tile([128, 7, C], BF16, tag="on_t")  # O^T normalized
            on_sb = sb.tile([128, 2, M], BF16, tag="on")      # O [c, m]
            y_sb = sb.tile([128, 2, M], F32, tag="y")
            warm = sb.tile([128, 1], F32, tag="warm")
            wu_in = sb.tile([128, 128], F32, tag="wu_in")

            # ---- input loads; k path first (longest pole) ----
            nc.sync.dma_start(out=wk_sb, in_=wk_d.rearrange("(kt p) (ot o) -> p kt ot o", p=128, o=128))
            nc.sync.dma_start(out=xf_sb[:, :, ds(0, MC)], in_=xf_r[:, :, ds(0, MC)])
            nc.sync.dma_start(out=xf_sb[:, :, ds(MC, MC)], in_=xf_r[:, :, ds(MC, MC)])
            nc.scalar.dma_start(out=xf_sb[:, :, ds(2 * MC, MC)], in_=xf_r[:, :, ds(2 * MC, MC)])
            nc.scalar.dma_start(out=xf_sb[:, :, ds(3 * MC, MC)], in_=xf_r[:, :, ds(3 * MC, MC)])
            nc.scalar.dma_start(out=wq_sb, in_=wq_d.rearrange("(kt p) (ot o) -> p kt ot o", p=128, o=128))
            nc.scalar.dma_start(out=wv_sb, in_=wv_d.rearrange("(kt p) c -> p kt c", p=128))
            nc.scalar.dma_start(out=wp_sb, in_=wp_d.rearrange("(kt p) (ot o) -> p kt ot o", p=128, o=128))
            nc.scalar.dma_start(out=b_sb, in_=b_d.rearrange("(ot p) one -> p ot one", p=128))

            # warm the exp table + ones columns while DMAs land
            nc.vector.memset(warm, 0.0)
            nc.scalar.activation(warm, warm, AF.Exp)
            nc.gpsimd.memset(vt_sb[:, :, :, 32:33], 1.0)
            nc.vector.memset(wu_in, 0.0)

            with tc.tile_pool(name="ps", bufs=1, space="PSUM") as ps:
                # hold the PE p-state through the initial DMA window
                def emit_wu(n):
                    for i in range(n):
                        wu = ps.tile([128, 2, MC], F32, tag="stA", name="wu", bufs=2)
                        nc.tensor.matmul(wu[:, 0, 0:128], lhsT=wu_in, rhs=wu_in, start=True, stop=True)

                emit_wu(10)

                # ---- phase-1 emitters (QKV projections, f32r) ----
                def kq_head(t_main, t3, g, h):
                    """Operand tile + column for head h of group g."""
                    if h < 3:
                        return t_main, ds(32 * h, 32), g
                    return t3, ds(0, 32), g

                def emit_k(ot, nck, eng):
                    pkt = ps.tile([128, 2, MC], F32, tag="stA", name="pk", bufs=2)
                    pk = pkt[:, 0, :]
                    for kt in range(2):
                        nc.tensor.matmul(
                            pk[:, 0:MC],
                            lhsT=wk_sb[:, kt, ot, :],
                            rhs=xf_sb[:, kt, ds(nck * MC, MC)],
                            start=(kt == 0),
                            stop=(kt == 1),
                        )
                    sl = ds(nck * MC, MC)
                    eng.tensor_copy(k_sb[:, ot, sl], pk[:, 0:MC])
                    if nck % 2 == 1:
                        # relocate head 3 (partition base 96) to a base-0 tile
                        hsl = ds((nck - 1) * MC, 2 * MC)
                        nc.sync.dma_start(out=k3_sb[:, ot, hsl], in_=k_sb[ds(96, 32), ot, hsl])

                def emit_q(ot, mc, eng):
                    pqt = ps.tile([128, 2, MC], F32, tag="stA", name="pq", bufs=2)
                    pq = pqt[:, 0, :]
                    for kt in range(2):
                        nc.tensor.matmul(
                            pq[:, 0:MC],
                            lhsT=wq_sb[:, kt, ot, :],
                            rhs=xf_sb[:, kt, ds(mc * MC, MC)],
                            start=(kt == 0),
                            stop=(kt == 1),
                        )
                    sl = ds(mc * MC, MC)
                    eng.tensor_copy(q_sb[:, ot, sl], pq[:, 0:MC])
                    nc.sync.dma_start(out=q3_sb[:, ot, sl], in_=q_sb[ds(96, 32), ot, sl])

                def emit_v(nt, eng):
                    """V^T[n-tile, all 256 c] in one go: x as stationary."""
                    w = NT_SIZES[nt]
                    pvt = ps.tile([128, 2, MC], F32, tag="stA", name="pv", bufs=2)
                    pv = pvt[:, 0, :]
                    for kt in range(2):
                        nc.tensor.matmul(
                            pv[:w, 0:C],
                            lhsT=xf_sb[:, kt, ds(nt * 128, w)],
                            rhs=wv_sb[:, kt, :],
                            start=(kt == 0),
                            stop=(kt == 1),
                        )
                    eng.tensor_copy(
                        vt_sb[:w, nt, :, 0:32],
                        pv[:w, 0:C].rearrange("p (h c) -> p h c", h=NH),
                    )

                # K/Q for head group 0 (ot=0) up front
                emit_k(0, 0, nc.vector)
                emit_q(0, 0, nc.gpsimd)
                emit_k(0, 1, nc.vector)
                emit_q(0, 1, nc.gpsimd)
                emit_k(0, 2, nc.vector)
                emit_k(0, 3, nc.gpsimd)
                emit_v(0, nc.vector)

                # ---- attention ----
                for g in range(2):
                    ot_ps = [
                        ps.tile([128, 7, 2, 33], F32, tag=f"ot{j}", name=f"ot{j}")
                        for j in range(2)
                    ]

                    def emit_st_act(nt, mc, ex):
                        """S^T + real exp for heads 0,1 (ACT): double-buffered
                        pair tile so the ACT chain pipelines across chunks."""
                        w = NT_SIZES[nt]
                        nsl = ds(nt * 128, w)
                        msl = ds(mc * MC, MC)
                        st = ps.tile([128, 2, MC], F32, tag="stA", name="stA", bufs=2)
                        for hh in range(2):
                            t, psl, col = kq_head(k_sb, k3_sb, g, hh)
                            tq, pslq, colq = kq_head(q_sb, q3_sb, g, hh)
                            nc.tensor.matmul(
                                st[:w, hh, 0:MC],
                                lhsT=t[psl, col, nsl],
                                rhs=tq[pslq, colq, msl],
                                start=True,
                                stop=True,
                            )
                        if nt == 13:
                            # last tile: bit-trick on DVE/Pool so the tail does
                            # not wait on two more long ACT exps
                            for hh, eng in ((0, nc.vector), (1, nc.gpsimd)):
                                eng.tensor_scalar(
                                    ex[:w, hh, msl].bitcast(I16),
                                    st[:w, hh, 0:MC],
                                    EXP_A, EXP_B, ALU.mult, ALU.add,
                                )
                        else:
                            nc.scalar.activation(ex[:w, 0:2, msl], st[:w, :, 0:MC], AF.Exp)

                    def emit_st_trick(nt, mc, h, eng, ex):
                        """S^T + bit-trick exp for head h (DVE or Pool)."""
                        w = NT_SIZES[nt]
                        nsl = ds(nt * 128, w)
                        msl = ds(mc * MC, MC)
                        st = ps.tile([128, MC], F32, tag=f"st{h}", name=f"st{h}", bufs=1)
                        t, psl, col = kq_head(k_sb, k3_sb, g, h)
                        tq, pslq, colq = kq_head(q_sb, q3_sb, g, h)
                        nc.tensor.matmul(
                            st[:w, 0:MC],
                            lhsT=t[psl, col, nsl],
                            rhs=tq[pslq, colq, msl],
                            start=True,
                            stop=True,
                        )
                        eng.tensor_scalar(
                            ex[:w, h, msl].bitcast(I16),
                            st[:w, 0:MC],
                            EXP_A, EXP_B, ALU.mult, ALU.add,
                        )

                    def emit_av(nt, ex, w, heads):
                        """A@V for the given heads of tile nt (one nt behind)."""
                        for h in heads:
                            for mt in range(7):
                                mw = MT_SIZES[mt]
                                nc.tensor.matmul(
                                    ot_ps[h // 2][:mw, mt, h % 2, 0:33],
                                    lhsT=ex[:w, h, ds(mt * 128, mw)],
                                    rhs=vt_sb[:w, nt, 4 * g + h, 0:33],
                                    start=(nt == 0 and mt == 0 and h % 2 == 0),
                                    stop=(nt == 13 and mt == 6 and h % 2 == 1),
                                )

                    prev_ex = None
                    prev_w = None
                    for nt in range(15):
                        ex = None
                        if nt < 14:
                            ex = rot.tile([128, 4, M], BF16, tag="expst", name="ex")
                            emit_st_act(nt, 0, ex)
                            emit_st_trick(nt, 0, 2, nc.gpsimd, ex)
                            emit_st_trick(nt, 0, 3, nc.vector, ex)
                        # AV h2/h3 first: their exp (DVE/Pool) lands earliest
                        if nt >= 1:
                            emit_av(nt - 1, prev_ex, prev_w, [2, 3])
                        # interleave group-1 QKV / V^T into group 0's loop
                        if g == 0:
                            if nt == 1:
                                emit_k(1, 0, nc.vector)
                            elif nt == 2:
                                emit_k(1, 1, nc.gpsimd)
                            elif nt == 3:
                                emit_k(1, 2, nc.vector)
                            elif nt == 4:
                                emit_k(1, 3, nc.gpsimd)
                            elif nt == 5:
                                emit_q(1, 0, nc.vector)
                            elif nt == 6:
                                emit_q(1, 1, nc.gpsimd)
                            if nt < 13:
                                emit_v(nt + 1, nc.gpsimd if nt % 2 else nc.vector)
                        if nt < 14:
                            emit_st_act(nt, 1, ex)
                            emit_st_trick(nt, 1, 2, nc.gpsimd, ex)
                            emit_st_trick(nt, 1, 3, nc.vector, ex)
                        if nt >= 1:
                            emit_av(nt - 1, prev_ex, prev_w, [0, 1])
                        if nt < 14:
                            prev_ex = ex
                            prev_w = NT_SIZES[nt]

                    # ---- normalize O^T (per-partition scale) into bf16 ----
                    def emit_proj(mc):
                        for ot in range(2):
                            yp = ps.tile([128, MC], F32, tag="st2" if ot else "st3", name="yp", bufs=1)
                            for g2 in range(2):
                                nc.tensor.matmul(
                                    yp[:, 0:MC],
                                    lhsT=wp_sb[:, g2, ot, :],
                                    rhs=on_sb[:, g2, ds(mc * MC, MC)],
                                    start=(g2 == 0),
                                    stop=(g2 == 1),
                                )
                            nc.vector.tensor_scalar_add(
                                y_sb[:, ot, ds(mc * MC, MC)], yp[:, 0:MC], b_sb[:, ot, :]
                            )

                    rs14s = []
                    for j in range(2):
                        rs14 = sb.tile([128, 7, 2], F32, tag=f"rs{g}{j}", name="rs14")
                        nc.vector.reciprocal(rs14, ot_ps[j][:, :, :, 32:33])
                        rs14s.append(rs14)
                    if g == 1:
                        # keep the PE p-state warm through the normalize window
                        for i in range(4):
                            wuT = ps.tile([128, MC], F32, tag="st2" if i % 2 else "st3", name="wut", bufs=1)
                            nc.tensor.matmul(wuT[:, 0:128], lhsT=wu_in, rhs=wu_in, start=True, stop=True)
                    engs = [nc.vector, nc.gpsimd, nc.scalar] if g == 1 else [nc.vector, nc.gpsimd]
                    ei = 0
                    for mt in range(7):
                        mw = MT_SIZES[mt]
                        for j in range(2):
                            for hh in range(2):
                                h = 2 * j + hh
                                eng = engs[ei % len(engs)]
                                ei += 1
                                if eng is nc.scalar:
                                    nc.scalar.activation(
                                        on_t_sb[:mw, mt, ds(32 * (4 * g + h), 32)],
                                        ot_ps[j][:mw, mt, hh, 0:32],
                                        AF.Copy,
                                        scale=rs14s[j][:mw, mt, hh : hh + 1],
                                    )
                                else:
                                    eng.tensor_scalar_mul(
                                        on_t_sb[:mw, mt, ds(32 * (4 * g + h), 32)],
                                        ot_ps[j][:mw, mt, hh, 0:32],
                                        rs14s[j][:mw, mt, hh : hh + 1],
                                    )
                        # O^T -> O via the DMA xbar transpose (off the PE)
                        dq = nc.sync if mt % 2 == 0 else nc.scalar
                        dq.dma_start_transpose(
                            on_sb[:, g, ds(mt * 128, mw)],
                            on_t_sb[:mw, mt, ds(g * 128, 128)],
                        )
                        # projection chunks as soon as their columns exist
                        if g == 1 and mt == 3:
                            emit_proj(0)
                        if g == 1 and mt == 6:
                            emit_proj(1)

                # ---- store (one DMA per output half) ----
                y_r = y_d.rearrange("(ot p) m -> ot p m", p=128)
                nc.sync.dma_start(out=y_r[0], in_=y_sb[:, 0, :])
                nc.scalar.dma_start(out=y_r[1], in_=y_sb[:, 1, :])

    nc.compile()
    return nc


_NC = None


def _get_nc():
    global _NC
    if _NC is None:
        _NC = build_program()
    return _NC


def make_in_maps(x, w_qkv, w_proj, b_proj):
    import ml_dtypes

    x = np.asarray(x, np.float32)
    w_qkv = np.asarray(w_qkv, np.float32)
    w_proj = np.asarray(w_proj, np.float32)
    b_proj = np.asarray(b_proj, np.float32)
    P = x.shape[0]
    xf = np.ascontiguousarray(x.reshape(P, C, N))
    wqT = np.ascontiguousarray((w_qkv[0:C] * SCALE).T)
    wkT = np.ascontiguousarray(w_qkv[C : 2 * C].T)
    wvT = np.ascontiguousarray(w_qkv[2 * C : 3 * C].T)
    wpT = np.ascontiguousarray(w_proj.T.astype(ml_dtypes.bfloat16))
    bias = np.ascontiguousarray(b_proj.reshape(C, 1))
    in_maps = []
    for core in range(N_CORES):
        p, mh = divmod(core, 2)
        if mh == 0:
            xp = xf[p]
        else:
            # rotate the n axis so this core's query half comes first
            xp = np.concatenate([xf[p][:, M:], xf[p][:, :M]], axis=1)
        in_maps.append(
            {
                "xf": np.ascontiguousarray(xp),
                "wqT": wqT,
                "wkT": wkT,
                "wvT": wvT,
                "wpT": wpT,
                "bias": bias,
            }
        )
    return in_maps


def assemble_output(per_core_y, x_shape):
    P, B, _, H, W, D = x_shape
    y = np.empty((P, C, N), np.float32)
    for core in range(N_CORES):
        p, mh = divmod(core, 2)
        y[p][:, mh * M : (mh + 1) * M] = per_core_y[core]
    return y.reshape(P, B, C, H, W, D)


def kernel(x, w_qkv, w_proj, b_proj):
    nc = _get_nc()
    in_maps = make_in_maps(x, w_qkv, w_proj, b_proj)
    res = run_bass_kernel_spmd(nc, in_maps, core_ids=list(range(N_CORES)))
    return assemble_output([res.results[c]["y"] for c in range(N_CORES)], x.shape)


# revision 36
# speedup vs baseline: 1.0232x; 1.0232x over previous
"""Trainium2 Bass kernel for nn_Attention_p_2757369004155.

Reference math (per p in 0..4):
  x [256, 1728] -> qkv = W_qkv @ x -> 8 heads of dim 32, N=1728
  attn = softmax((q*scale)^T k), out = v @ attn^T, y = W_p @ out + b

Sharding: 8 cores = 4 p-branches x 2 query-halves. Each core is fully
self-contained (K/V computed for all n, Q for its half). The host permutes
each core's n axis so its query half is always columns [0, 864).

Engine budget per core (cost-model cycles at 2.4 GHz):
  - QKV + projection matmuls run as f32r (1 cycle/row at ap>=256, vs fp32's
    4): K/Q/V/proj ~ 8.8 us total on PE.
  - S^T (QK^T) runs per-head as single-pass f32r (contraction 32, no
    residual stack): 8 heads x 14 n-tiles x 864 m ~ 40 us. Per-head operand
    slices address k_sb/q_sb partitions directly, so no stack-building DMAs.
  - exp(S^T) is the largest elementwise load (11.9M elements). It is split
    across three engines: ACT computes real exp (PSUM->SBUF bf16), DVE and
    Pool compute a Schraudolph bit-trick exp (one tensor_scalar op:
    i16 = x*128/ln2 + 16252.5 viewed as bf16), which is accurate to ~2-3%
    per element and dilutes to <1e-3 after softmax averaging (validated in
    numpy against the reference; tolerance is 2e-2).
  - A@V runs in the O^T[m, c] orientation with bf16 operands (1 cycle/row):
    exp tiles as stationary, V^T (with a ones column for the softmax
    denominators) as moving: ~11 us on PE.
  - Softmax normalization is a per-partition scaled PSUM->SBUF copy into
    bf16 O^T tiles; the O^T -> O transpose uses the DMA xbar transpose
    (16x128 tiles, off the PE entirely); the final projection is bf16.
"""

import math

import numpy as np

import concourse.bass as bass
import concourse.tile as tile
from concourse import bacc, mybir
from concourse.bass import ds, broadcast_tensor_aps
from concourse.bass_utils import run_bass_kernel_spmd
from concourse.masks import make_identity

F32 = mybir.dt.float32
F32R = mybir.dt.float32r
BF16 = mybir.dt.bfloat16
I16 = mybir.dt.int16
AF = mybir.ActivationFunctionType
ALU = mybir.AluOpType

N_CORES = 8
C = 256            # channels
NH = 8             # heads
HD = 32            # head dim
N = 1728           # sequence (12*12*12)
M = N // 2         # per-core query positions
MC = 432           # m chunk (PSUM bank = 512 fp32)
NT_SIZES = [128] * 13 + [64]          # n contraction tiles
MT_SIZES = [128] * 6 + [96]           # m tiles for the O^T matmuls
SCALE = HD ** -0.5

# Schraudolph exp in bf16-bit space: i16 = round(x * 128/ln2 + B).
# B = 127*128 - C + 0.5; C=4 balances the piecewise-linear 2^frac error.
EXP_A = 128.0 / math.log(2.0)
EXP_B = 127.0 * 128.0 - 4.0 + 0.5


def build_program():
    nc = bacc.Bacc(
        "TRN2",
        target_bir_lowering=False,
        debug=False,
        enable_asserts=False,
        num_devices=N_CORES,
    )

    xf_d = nc.dram_tensor("xf", [C, N], F32R, kind="ExternalInput").ap()
    wq_d = nc.dram_tensor("wqT", [C, C], F32R, kind="ExternalInput").ap()
    wk_d = nc.dram_tensor("wkT", [C, C], F32R, kind="ExternalInput").ap()
    wv_d = nc.dram_tensor("wvT", [C, C], F32R, kind="ExternalInput").ap()
    wp_d = nc.dram_tensor("wpT", [C, C], BF16, kind="ExternalInput").ap()
    b_d = nc.dram_tensor("bias", [C, 1], F32, kind="ExternalInput").ap()
    y_d = nc.dram_tensor("y", [C, M], F32, kind="ExternalOutput").ap()

    xf_r = xf_d.rearrange("(kt p) n -> p kt n", p=128)

    with tile.TileContext(nc) as tc:
        with (
            tc.tile_pool(name="persist", bufs=1) as sb,
            tc.tile_pool(name="rot", bufs=3) as rot,
        ):
            # ---- persistent SBUF tiles ----
            wk_sb = sb.tile([128, 2, 2, 128], F32R, tag="wk")
            wq_sb = sb.tile([128, 2, 2, 128], F32R, tag="wq")
            wv_sb = sb.tile([128, 2, C], F32R, tag="wv")
            wp_sb = sb.tile([128, 2, 2, 128], BF16, tag="wp")
            b_sb = sb.tile([128, 2, 1], F32, tag="b")
            xf_sb = sb.tile([128, 2, N], F32R, tag="xf")
            k_sb = sb.tile([128, 2, N], F32R, tag="k")
            q_sb = sb.tile([128, 2, M], F32R, tag="q")
            # PE operands must sit at partition base 0/32/64: heads 0-2 of
            # each group are addressed directly in k_sb/q_sb; head 3 (base
            # 96) is DMA-relocated to these base-0 tiles.
            k3_sb = sb.tile([32, 2, N], F32R, tag="k3")
            q3_sb = sb.tile([32, 2, M], F32R, tag="q3")
            # V^T tiles with a ones column per head ([n, nt, head, 33]);
            # col 33 is alignment padding.
            vt_sb = sb.tile([128, 14, NH, 34], BF16, tag="vt")
            on_t_sb = sb.tile([128, 7, C], BF16, tag="on_t")  # O^T normalized
            on_sb = sb.tile([128, 2, M], BF16, tag="on")      # O [c, m]
            y_sb = sb.tile([128, 2, M], F32, tag="y")
            warm = sb.tile([128, 1], F32, tag="warm")
            wu_in = sb.tile([128, 128], F32, tag="wu_in")

            # ---- input loads; k path first (longest pole) ----
            nc.sync.dma_start(out=wk_sb, in_=wk_d.rearrange("(kt p) (ot o) -> p kt ot o", p=128, o=128))
            nc.sync.dma_start(out=xf_sb[:, :, ds(0, MC)], in_=xf_r[:, :, ds(0, MC)])
            nc.sync.dma_start(out=xf_sb[:, :, ds(MC, MC)], in_=xf_r[:, :, ds(MC, MC)])
            nc.scalar.dma_start(out=wq_sb, in_=wq_d.rearrange("(kt p) (ot o) -> p kt ot o", p=128, o=128))
            nc.scalar.dma_start(out=xf_sb[:, :, ds(2 * MC, MC)], in_=xf_r[:, :, ds(2 * MC, MC)])
            nc.scalar.dma_start(out=xf_sb[:, :, ds(3 * MC, MC)], in_=xf_r[:, :, ds(3 * MC, MC)])
            nc.scalar.dma_start(out=wv_sb, in_=wv_d.rearrange("(kt p) c -> p kt c", p=128))

            # warm the exp table + ones columns while DMAs land
            nc.vector.memset(warm, 0.0)
            nc.scalar.activation(warm, warm, AF.Exp)
            nc.gpsimd.memset(vt_sb[:, :, :, 32:33], 1.0)
            nc.vector.memset(wu_in, 0.0)
            ident = sb.tile([128, 128], BF16, tag="ident")
            make_identity(nc, ident)

            with tc.tile_pool(name="ps", bufs=1, space="PSUM") as ps:
                # hold the PE p-state through the initial DMA window
                def emit_wu(n):
                    for i in range(n):
                        wu = ps.tile([128, 2, MC], F32, tag="stA", name="wu", bufs=2)
                        nc.tensor.matmul(wu[:, 0, 0:128], lhsT=wu_in, rhs=wu_in, start=True, stop=True)

                emit_wu(10)

                # ---- phase-1 emitters (QKV projections, f32r) ----
                def kq_head(t_main, t3, g, h):
                    """Operand tile + column for head h of group g."""
                    if h < 3:
                        return t_main, ds(32 * h, 32), g
                    return t3, ds(0, 32), g

                def emit_k(ot, nck, eng):
                    pkt = ps.tile([128, 2, MC], F32, tag="stA", name="pk", bufs=2)
                    pk = pkt[:, 0, :]
                    for kt in range(2):
                        nc.tensor.matmul(
                            pk[:, 0:MC],
                            lhsT=wk_sb[:, kt, ot, :],
                            rhs=xf_sb[:, kt, ds(nck * MC, MC)],
                            start=(kt == 0),
                            stop=(kt == 1),
                        )
                    sl = ds(nck * MC, MC)
                    eng.tensor_copy(k_sb[:, ot, sl], pk[:, 0:MC])
                    if nck % 2 == 1:
                        # relocate head 3 (partition base 96) to a base-0 tile
                        hsl = ds((nck - 1) * MC, 2 * MC)
                        nc.sync.dma_start(out=k3_sb[:, ot, hsl], in_=k_sb[ds(96, 32), ot, hsl])

                def emit_q(ot, mc, eng):
                    pqt = ps.tile([128, 2, MC], F32, tag="stA", name="pq", bufs=2)
                    pq = pqt[:, 0, :]
                    for kt in range(2):
                        nc.tensor.matmul(
                            pq[:, 0:MC],
                            lhsT=wq_sb[:, kt, ot, :],
                            rhs=xf_sb[:, kt, ds(mc * MC, MC)],
                            start=(kt == 0),
                            stop=(kt == 1),
                        )
                    sl = ds(mc * MC, MC)
                    eng.tensor_copy(q_sb[:, ot, sl], pq[:, 0:MC])
                    nc.sync.dma_start(out=q3_sb[:, ot, sl], in_=q_sb[ds(96, 32), ot, sl])

                def emit_v(nt, eng):
                    """V^T[n-tile, all 256 c] in one go: x as stationary."""
                    w = NT_SIZES[nt]
                    pvt = ps.tile([128, 2, MC], F32, tag="stA", name="pv", bufs=2)
                    pv = pvt[:, 0, :]
                    for kt in range(2):
                        nc.tensor.matmul(
                            pv[:w, 0:C],
                            lhsT=xf_sb[:, kt, ds(nt * 128, w)],
                            rhs=wv_sb[:, kt, :],
                            start=(kt == 0),
                            stop=(kt == 1),
                        )
                    eng.tensor_copy(
                        vt_sb[:w, nt, :, 0:32],
                        pv[:w, 0:C].rearrange("p (h c) -> p h c", h=NH),
                    )

                # K/Q for head group 0 (ot=0) up front
                emit_k(0, 0, nc.vector)
                emit_q(0, 0, nc.gpsimd)
                emit_k(0, 1, nc.vector)
                emit_q(0, 1, nc.gpsimd)
                emit_k(0, 2, nc.vector)
                emit_k(0, 3, nc.gpsimd)
                emit_v(0, nc.vector)

                # ---- attention ----
                for g in range(2):
                    ot_ps = [
                        ps.tile([128, 7, 2, 33], F32, tag=f"ot{j}", name=f"ot{j}")
                        for j in range(2)
                    ]

                    def emit_st_act(nt, mc, ex):
                        """S^T + real exp for heads 0,1 (ACT): double-buffered
                        pair tile so the ACT chain pipelines across chunks."""
                        w = NT_SIZES[nt]
                        nsl = ds(nt * 128, w)
                        msl = ds(mc * MC, MC)
                        st = ps.tile([128, 2, MC], F32, tag="stA", name="stA", bufs=2)
                        for hh in range(2):
                            t, psl, col = kq_head(k_sb, k3_sb, g, hh)
                            tq, pslq, colq = kq_head(q_sb, q3_sb, g, hh)
                            nc.tensor.matmul(
                                st[:w, hh, 0:MC],
                                lhsT=t[psl, col, nsl],
                                rhs=tq[pslq, colq, msl],
                                start=True,
                                stop=True,
                            )
                        if nt == 13:
                            # last tile: bit-trick on DVE/Pool so the tail does
                            # not wait on two more long ACT exps
                            for hh, eng in ((0, nc.vector), (1, nc.gpsimd)):
                                eng.tensor_scalar(
                                    ex[:w, hh, msl].bitcast(I16),
                                    st[:w, hh, 0:MC],
                                    EXP_A, EXP_B, ALU.mult, ALU.add,
                                )
                        else:
                            nc.scalar.activation(ex[:w, 0:2, msl], st[:w, :, 0:MC], AF.Exp)

                    def emit_st_trick(nt, mc, h, eng, ex):
                        """S^T + bit-trick exp for head h (DVE or Pool)."""
                        w = NT_SIZES[nt]
                        nsl = ds(nt * 128, w)
                        msl = ds(mc * MC, MC)
                        st = ps.tile([128, MC], F32, tag=f"st{h}", name=f"st{h}", bufs=1)
                        t, psl, col = kq_head(k_sb, k3_sb, g, h)
                        tq, pslq, colq = kq_head(q_sb, q3_sb, g, h)
                        nc.tensor.matmul(
                            st[:w, 0:MC],
                            lhsT=t[psl, col, nsl],
                            rhs=tq[pslq, colq, msl],
                            start=True,
                            stop=True,
                        )
                        eng.tensor_scalar(
                            ex[:w, h, msl].bitcast(I16),
                            st[:w, 0:MC],
                            EXP_A, EXP_B, ALU.mult, ALU.add,
                        )

                    def emit_av(nt, ex, w, heads):
                        """A@V for the given heads of tile nt (one nt behind)."""
                        for h in heads:
                            for mt in range(7):
                                mw = MT_SIZES[mt]
                                nc.tensor.matmul(
                                    ot_ps[h // 2][:mw, mt, h % 2, 0:33],
                                    lhsT=ex[:w, h, ds(mt * 128, mw)],
                                    rhs=vt_sb[:w, nt, 4 * g + h, 0:33],
                                    start=(nt == 0 and mt == 0 and h % 2 == 0),
                                    stop=(nt == 13 and mt == 6 and h % 2 == 1),
                                )

                    prev_ex = None
                    prev_w = None
                    for nt in range(15):
                        ex = None
                        if nt < 14:
                            ex = rot.tile([128, 4, M], BF16, tag="expst", name="ex")
                            emit_st_act(nt, 0, ex)
                            emit_st_trick(nt, 0, 2, nc.gpsimd, ex)
                            emit_st_trick(nt, 0, 3, nc.vector, ex)
                        # AV h2/h3 first: their exp (DVE/Pool) lands earliest
                        if nt >= 1:
                            emit_av(nt - 1, prev_ex, prev_w, [2, 3])
                        # interleave group-1 QKV / V^T into group 0's loop
                        if g == 0:
                            if nt == 1:
                                emit_k(1, 0, nc.vector)
                            elif nt == 2:
                                emit_k(1, 1, nc.gpsimd)
                            elif nt == 3:
                                emit_k(1, 2, nc.vector)
                            elif nt == 4:
                                emit_k(1, 3, nc.gpsimd)
                            elif nt == 5:
                                emit_q(1, 0, nc.vector)
                            elif nt == 6:
                                emit_q(1, 1, nc.gpsimd)
                            elif nt == 7:
                                nc.scalar.dma_start(out=wp_sb, in_=wp_d.rearrange("(kt p) (ot o) -> p kt ot o", p=128, o=128))
                                nc.scalar.dma_start(out=b_sb, in_=b_d.rearrange("(ot p) one -> p ot one", p=128))
                            if nt < 13:
                                emit_v(nt + 1, nc.gpsimd if nt % 2 else nc.vector)
                        if nt < 14:
                            emit_st_act(nt, 1, ex)
                            emit_st_trick(nt, 1, 2, nc.gpsimd, ex)
                            emit_st_trick(nt, 1, 3, nc.vector, ex)
                        if nt >= 1:
                            emit_av(nt - 1, prev_ex, prev_w, [0, 1])
                        if nt < 14:
                            prev_ex = ex
                            prev_w = NT_SIZES[nt]

                    # ---- normalize O^T (per-partition scale) into bf16 ----
                    def emit_proj(mc):
                        for ot in range(2):
                            yp = ps.tile([128, MC], F32, tag="st2" if ot else "st3", name="yp", bufs=1)
                            for g2 in range(2):
                                nc.tensor.matmul(
                                    yp[:, 0:MC],
                                    lhsT=wp_sb[:, g2, ot, :],
                                    rhs=on_sb[:, g2, ds(mc * MC, MC)],
                                    start=(g2 == 0),
                                    stop=(g2 == 1),
                                )
                            (nc.vector if ot == 0 else nc.gpsimd).tensor_scalar_add(
                                y_sb[:, ot, ds(mc * MC, MC)], yp[:, 0:MC], b_sb[:, ot, :]
                            )

                    # ---- normalize O^T: one fused broadcast-multiply per
                    # head pair (PSUM -> bf16 SBUF), then transpose ----
                    for j, eng in ((0, nc.vector), (1, nc.gpsimd)):
                        rs14 = sb.tile([128, 7, 2, 1], F32, tag=f"rs{g}{j}", name="rs14")
                        nc.vector.reciprocal(rs14, ot_ps[j][:, :, :, 32:33])
                        out_v = on_t_sb[:, :, ds(128 * g + 64 * j, 64)].rearrange(
                            "p mt (hh c) -> p mt hh c", hh=2
                        )
                        in0, in1 = broadcast_tensor_aps(ot_ps[j][:, :, :, 0:32], rs14)
                        eng.tensor_tensor(out_v, in0, in1, op=ALU.mult)
                    if g == 1:
                        # keep the PE p-state warm through the normalize window
                        for i in range(4):
                            wuT = ps.tile([128, MC], F32, tag="st2" if i % 2 else "st3", name="wut", bufs=1)
                            nc.tensor.matmul(wuT[:, 0:128], lhsT=wu_in, rhs=wu_in, start=True, stop=True)
                    for mt in range(7):
                        mw = MT_SIZES[mt]
                        if g == 0:
                            # O^T -> O via the DMA xbar transpose (HWDGE is
                            # idle mid-kernel)
                            dq = nc.sync if mt % 2 == 0 else nc.scalar
                            dq.dma_start_transpose(
                                on_sb[:, g, ds(mt * 128, mw)],
                                on_t_sb[:mw, mt, ds(g * 128, 128)],
                            )
                        else:
                            # tail: PE transpose (bf16) + spread PSUM->SBUF copies
                            tp = ps.tile([128, 128], BF16, tag="st2" if mt % 2 else "st3", name="tp", bufs=1)
                            nc.tensor.transpose(
                                tp[:, :mw],
                                on_t_sb[:mw, mt, ds(g * 128, 128)],
                                ident[:mw, :mw],
                            )
                            ceng = (nc.vector, nc.gpsimd, nc.scalar)[mt % 3]
                            if ceng is nc.scalar:
                                nc.scalar.activation(
                                    on_sb[:, g, ds(mt * 128, mw)], tp[:, :mw], AF.Copy
                                )
                            else:
                                ceng.tensor_copy(on_sb[:, g, ds(mt * 128, mw)], tp[:, :mw])
                            # projection chunks as soon as their columns exist
                            if mt == 3:
                                emit_proj(0)
                            if mt == 6:
                                emit_proj(1)

                # ---- store (one DMA per output half) ----
                y_r = y_d.rearrange("(ot p) m -> ot p m", p=128)
                nc.sync.dma_start(out=y_r[0], in_=y_sb[:, 0, :])
                nc.scalar.dma_start(out=y_r[1], in_=y_sb[:, 1, :])

    nc.compile()
    return nc


_NC = None


def _get_nc():
    global _NC
    if _NC is None:
        _NC = build_program()
    return _NC


def make_in_maps(x, w_qkv, w_proj, b_proj):
    import ml_dtypes

    x = np.asarray(x, np.float32)
    w_qkv = np.asarray(w_qkv, np.float32)
    w_proj = np.asarray(w_proj, np.float32)
    b_proj = np.asarray(b_proj, np.float32)
    P = x.shape[0]
    xf = np.ascontiguousarray(x.reshape(P, C, N))
    wqT = np.ascontiguousarray((w_qkv[0:C] * SCALE).T)
    wkT = np.ascontiguousarray(w_qkv[C : 2 * C].T)
    wvT = np.ascontiguousarray(w_qkv[2 * C : 3 * C].T)
    wpT = np.ascontiguousarray(w_proj.T.astype(ml_dtypes.bfloat16))
    bias = np.ascontiguousarray(b_proj.reshape(C, 1))
    in_maps = []
    for core in range(N_CORES):
        p, mh = divmod(core, 2)
        if mh == 0:
            xp = xf[p]
        else:
            # rotate the n axis so this core's query half comes first
            xp = np.concatenate([xf[p][:, M:], xf[p][:, :M]], axis=1)
        in_maps.append(
            {
                "xf": np.ascontiguousarray(xp),
                "wqT": wqT,
                "wkT": wkT,
                "wvT": wvT,
                "wpT": wpT,
                "bias": bias,
            }
        )
    return in_maps


def assemble_output(per_core_y, x_shape):
    P, B, _, H, W, D = x_shape
    y = np.empty((P, C, N), np.float32)
    for core in range(N_CORES):
        p, mh = divmod(core, 2)
        y[p][:, mh * M : (mh + 1) * M] = per_core_y[core]
    return y.reshape(P, B, C, H, W, D)


def kernel(x, w_qkv, w_proj, b_proj):
    nc = _get_nc()
    in_maps = make_in_maps(x, w_qkv, w_proj, b_proj)
    res = run_bass_kernel_spmd(nc, in_maps, core_ids=list(range(N_CORES)))
    return assemble_output([res.results[c]["y"] for c in range(N_CORES)], x.shape)


# revision 39
# speedup vs baseline: 1.0918x; 1.0671x over previous
"""Trainium2 Bass kernel for nn_Attention_p_2757369004155.

Reference math (per p in 0..4):
  x [256, 1728] -> qkv = W_qkv @ x -> 8 heads of dim 32, N=1728
  attn = softmax((q*scale)^T k), out = v @ attn^T, y = W_p @ out + b

Sharding: 8 cores = 4 p-branches x 2 query-halves. Each core is fully
self-contained (K/V computed for all n, Q for its half). The host permutes
each core's n axis so its query half is always columns [0, 864).

Engine budget per core (cost-model cycles at 2.4 GHz):
  - QKV + projection matmuls run as f32r (1 cycle/row at ap>=256, vs fp32's
    4): K/Q/V/proj ~ 8.8 us total on PE.
  - S^T (QK^T) runs per-head as single-pass f32r (contraction 32, no
    residual stack): 8 heads x 14 n-tiles x 864 m ~ 40 us. Per-head operand
    slices address k_sb/q_sb partitions directly, so no stack-building DMAs.
  - exp(S^T) is the largest elementwise load (11.9M elements). It is split
    across three engines: ACT computes real exp (PSUM->SBUF bf16), DVE and
    Pool compute a Schraudolph bit-trick exp (one tensor_scalar op:
    i16 = x*128/ln2 + 16252.5 viewed as bf16), which is accurate to ~2-3%
    per element and dilutes to <1e-3 after softmax averaging (validated in
    numpy against the reference; tolerance is 2e-2).
  - A@V runs in the O^T[m, c] orientation with bf16 operands (1 cycle/row):
    exp tiles as stationary, V^T (with a ones column for the softmax
    denominators) as moving: ~11 us on PE.
  - Softmax normalization is a per-partition scaled PSUM->SBUF copy into
    bf16 O^T tiles; the O^T -> O transpose uses the DMA xbar transpose
    (16x128 tiles, off the PE entirely); the final projection is bf16.
"""

import math

import numpy as np

import concourse.bass as bass
import concourse.tile as tile
from concourse import bacc, mybir
from concourse.bass import ds, broadcast_tensor_aps
from concourse.bass_utils import run_bass_kernel_spmd
from concourse.masks import make_identity

F32 = mybir.dt.float32
F32R = mybir.dt.float32r
BF16 = mybir.dt.bfloat16
I16 = mybir.dt.int16
AF = mybir.ActivationFunctionType
ALU = mybir.AluOpType

N_CORES = 8
C = 256            # channels
NH = 8             # heads
HD = 32            # head dim
N = 1728           # sequence (12*12*12)
M = N // 2         # per-core query positions
MC = 432           # m chunk (PSUM bank = 512 fp32)
NT_SIZES = [128] * 13 + [64]          # n contraction tiles
MT_SIZES = [128] * 6 + [96]           # m tiles for the O^T matmuls
SCALE = HD ** -0.5

# Schraudolph exp in bf16-bit space: i16 = round(x * 128/ln2 + B).
# B = 127*128 - C + 0.5; C=4 balances the piecewise-linear 2^frac error.
EXP_A = 128.0 / math.log(2.0)
EXP_B = 127.0 * 128.0 - 4.0 + 0.5


def build_program():
    nc = bacc.Bacc(
        "TRN2",
        target_bir_lowering=False,
        debug=False,
        enable_asserts=False,
        num_devices=N_CORES,
    )

    xf_d = nc.dram_tensor("xf", [C, N], F32R, kind="ExternalInput").ap()
    wq_d = nc.dram_tensor("wqT", [C, C], F32R, kind="ExternalInput").ap()
    wk_d = nc.dram_tensor("wkT", [C, C], F32R, kind="ExternalInput").ap()
    wv_d = nc.dram_tensor("wvT", [C, C], F32R, kind="ExternalInput").ap()
    wp_d = nc.dram_tensor("wpT", [C, C], BF16, kind="ExternalInput").ap()
    b_d = nc.dram_tensor("bias", [C, 1], F32, kind="ExternalInput").ap()
    y_d = nc.dram_tensor("y", [C, M], F32, kind="ExternalOutput").ap()

    xf_r = xf_d.rearrange("(kt p) n -> p kt n", p=128)

    with tile.TileContext(nc) as tc:
        with (
            tc.tile_pool(name="persist", bufs=1) as sb,
            tc.tile_pool(name="rot", bufs=3) as rot,
        ):
            # ---- persistent SBUF tiles ----
            wk_sb = sb.tile([128, 2, 2, 128], F32R, tag="wk")
            wq_sb = sb.tile([128, 2, 2, 128], F32R, tag="wq")
            wv_sb = sb.tile([128, 2, C], F32R, tag="wv")
            wp_sb = sb.tile([128, 2, 2, 128], BF16, tag="wp")
            b_sb = sb.tile([128, 2, 1], F32, tag="b")
            xf_sb = sb.tile([128, 2, N], F32R, tag="xf")
            k_sb = sb.tile([128, 2, N], F32R, tag="k")
            q_sb = sb.tile([128, 2, M], F32R, tag="q")
            # PE operands must sit at partition base 0/32/64: heads 0-2 of
            # each group are addressed directly in k_sb/q_sb; head 3 (base
            # 96) is DMA-relocated to these base-0 tiles.
            k3_sb = sb.tile([32, 2, N], F32R, tag="k3")
            q3_sb = sb.tile([32, 2, M], F32R, tag="q3")
            # V^T tiles with a ones column per head ([n, nt, head, 33]);
            # col 33 is alignment padding.
            vt_sb = sb.tile([128, 14, NH, 34], BF16, tag="vt")
            on_t_sb = sb.tile([128, 7, C], BF16, tag="on_t")  # O^T normalized
            on_sb = sb.tile([128, 2, M], BF16, tag="on")      # O [c, m]
            y_sb = sb.tile([128, 2, M], F32, tag="y")
            warm = sb.tile([128, 1], F32, tag="warm")
            wu_in = sb.tile([128, 128], F32, tag="wu_in")

            # ---- input loads; k path first (longest pole) ----
            nc.sync.dma_start(out=wk_sb, in_=wk_d.rearrange("(kt p) (ot o) -> p kt ot o", p=128, o=128))
            nc.sync.dma_start(out=xf_sb[:, :, ds(0, MC)], in_=xf_r[:, :, ds(0, MC)])
            nc.sync.dma_start(out=xf_sb[:, :, ds(MC, MC)], in_=xf_r[:, :, ds(MC, MC)])
            nc.scalar.dma_start(out=wq_sb, in_=wq_d.rearrange("(kt p) (ot o) -> p kt ot o", p=128, o=128))
            nc.scalar.dma_start(out=xf_sb[:, :, ds(2 * MC, MC)], in_=xf_r[:, :, ds(2 * MC, MC)])
            nc.scalar.dma_start(out=xf_sb[:, :, ds(3 * MC, MC)], in_=xf_r[:, :, ds(3 * MC, MC)])
            nc.scalar.dma_start(out=wv_sb, in_=wv_d.rearrange("(kt p) c -> p kt c", p=128))

            # warm the exp table + ones columns while DMAs land
            nc.vector.memset(warm, 0.0)
            nc.scalar.activation(warm, warm, AF.Exp)
            nc.gpsimd.memset(vt_sb[:, :, :, 32:33], 1.0)
            nc.vector.memset(wu_in, 0.0)
            ident = sb.tile([128, 128], BF16, tag="ident")
            make_identity(nc, ident)

            with tc.tile_pool(name="ps", bufs=1, space="PSUM") as ps:
                # hold the PE p-state through the initial DMA window
                def emit_wu(n):
                    for i in range(n):
                        wu = ps.tile([128, 2, MC], F32, tag="stA", name="wu", bufs=2)
                        nc.tensor.matmul(wu[:, 0, 0:128], lhsT=wu_in, rhs=wu_in, start=True, stop=True)

                emit_wu(10)

                # ---- phase-1 emitters (QKV projections, f32r) ----
                def kq_head(t_main, t3, g, h):
                    """Operand tile + column for head h of group g."""
                    if h < 3:
                        return t_main, ds(32 * h, 32), g
                    return t3, ds(0, 32), g

                def emit_k(ot, nck, eng):
                    pkt = ps.tile([128, 2, MC], F32, tag="stA", name="pk", bufs=2)
                    pk = pkt[:, 0, :]
                    for kt in range(2):
                        nc.tensor.matmul(
                            pk[:, 0:MC],
                            lhsT=wk_sb[:, kt, ot, :],
                            rhs=xf_sb[:, kt, ds(nck * MC, MC)],
                            start=(kt == 0),
                            stop=(kt == 1),
                        )
                    sl = ds(nck * MC, MC)
                    eng.tensor_copy(k_sb[:, ot, sl], pk[:, 0:MC])
                    if nck % 2 == 1:
                        # relocate head 3 (partition base 96) to a base-0 tile
                        hsl = ds((nck - 1) * MC, 2 * MC)
                        nc.sync.dma_start(out=k3_sb[:, ot, hsl], in_=k_sb[ds(96, 32), ot, hsl])

                def emit_q(ot, mc, eng):
                    pqt = ps.tile([128, 2, MC], F32, tag="stA", name="pq", bufs=2)
                    pq = pqt[:, 0, :]
                    for kt in range(2):
                        nc.tensor.matmul(
                            pq[:, 0:MC],
                            lhsT=wq_sb[:, kt, ot, :],
                            rhs=xf_sb[:, kt, ds(mc * MC, MC)],
                            start=(kt == 0),
                            stop=(kt == 1),
                        )
                    sl = ds(mc * MC, MC)
                    eng.tensor_copy(q_sb[:, ot, sl], pq[:, 0:MC])
                    nc.sync.dma_start(out=q3_sb[:, ot, sl], in_=q_sb[ds(96, 32), ot, sl])

                def emit_v(nt, eng):
                    """V^T[n-tile, all 256 c] in one go: x as stationary."""
                    w = NT_SIZES[nt]
                    pvt = ps.tile([128, 2, MC], F32, tag="stA", name="pv", bufs=2)
                    pv = pvt[:, 0, :]
                    for kt in range(2):
                        nc.tensor.matmul(
                            pv[:w, 0:C],
                            lhsT=xf_sb[:, kt, ds(nt * 128, w)],
                            rhs=wv_sb[:, kt, :],
                            start=(kt == 0),
                            stop=(kt == 1),
                        )
                    eng.tensor_copy(
                        vt_sb[:w, nt, :, 0:32],
                        pv[:w, 0:C].rearrange("p (h c) -> p h c", h=NH),
                    )

                # K/Q for head group 0 (ot=0) up front
                emit_k(0, 0, nc.vector)
                emit_q(0, 0, nc.gpsimd)
                emit_k(0, 1, nc.vector)
                emit_q(0, 1, nc.gpsimd)
                emit_k(0, 2, nc.vector)
                emit_k(0, 3, nc.gpsimd)
                emit_v(0, nc.vector)

                # ---- attention ----
                for g in range(2):
                    ot_ps = [
                        ps.tile([128, 7, 2, 33], F32, tag=f"ot{j}", name=f"ot{j}")
                        for j in range(2)
                    ]

                    def emit_st_act(nt, mc, ex):
                        """S^T + real exp for heads 0,1 (ACT): double-buffered
                        pair tile so the ACT chain pipelines across chunks."""
                        w = NT_SIZES[nt]
                        nsl = ds(nt * 128, w)
                        msl = ds(mc * MC, MC)
                        st = ps.tile([128, 2, MC], F32, tag="stA", name="stA", bufs=2)
                        for hh in range(2):
                            t, psl, col = kq_head(k_sb, k3_sb, g, hh)
                            tq, pslq, colq = kq_head(q_sb, q3_sb, g, hh)
                            nc.tensor.matmul(
                                st[:w, hh, 0:MC],
                                lhsT=t[psl, col, nsl],
                                rhs=tq[pslq, colq, msl],
                                start=True,
                                stop=True,
                            )
                        if nt == 13:
                            # last tile: bit-trick on DVE/Pool so the tail does
                            # not wait on two more long ACT exps
                            for hh, eng in ((0, nc.vector), (1, nc.gpsimd)):
                                eng.tensor_scalar(
                                    ex[:w, hh, msl].bitcast(I16),
                                    st[:w, hh, 0:MC],
                                    EXP_A, EXP_B, ALU.mult, ALU.add,
                                )
                        else:
                            nc.scalar.activation(ex[:w, 0:2, msl], st[:w, :, 0:MC], AF.Exp)

                    def emit_st_trick(nt, mc, h, eng, ex):
                        """S^T + bit-trick exp for head h (DVE or Pool)."""
                        w = NT_SIZES[nt]
                        nsl = ds(nt * 128, w)
                        msl = ds(mc * MC, MC)
                        st = ps.tile([128, MC], F32, tag=f"st{h}", name=f"st{h}", bufs=1)
                        t, psl, col = kq_head(k_sb, k3_sb, g, h)
                        tq, pslq, colq = kq_head(q_sb, q3_sb, g, h)
                        nc.tensor.matmul(
                            st[:w, 0:MC],
                            lhsT=t[psl, col, nsl],
                            rhs=tq[pslq, colq, msl],
                            start=True,
                            stop=True,
                        )
                        eng.tensor_scalar(
                            ex[:w, h, msl].bitcast(I16),
                            st[:w, 0:MC],
                            EXP_A, EXP_B, ALU.mult, ALU.add,
                        )

                    def emit_av(nt, ex, w, heads):
                        """A@V for the given heads of tile nt (one nt behind)."""
                        for h in heads:
                            for mt in range(7):
                                mw = MT_SIZES[mt]
                                nc.tensor.matmul(
                                    ot_ps[h // 2][:mw, mt, h % 2, 0:33],
                                    lhsT=ex[:w, h, ds(mt * 128, mw)],
                                    rhs=vt_sb[:w, nt, 4 * g + h, 0:33],
                                    start=(nt == 0 and mt == 0 and h % 2 == 0),
                                    stop=(nt == 13 and mt == 6 and h % 2 == 1),
                                )

                    prev_ex = None
                    prev_w = None
                    for nt in range(15):
                        ex = None
                        if nt < 14:
                            ex = rot.tile([128, 4, M], BF16, tag="expst", name="ex")
                            emit_st_act(nt, 0, ex)
                            emit_st_trick(nt, 0, 2, nc.gpsimd, ex)
                            emit_st_trick(nt, 0, 3, nc.vector, ex)
                        # AV h2/h3 first: their exp (DVE/Pool) lands earliest
                        if nt >= 1:
                            emit_av(nt - 1, prev_ex, prev_w, [2, 3])
                        # interleave group-1 QKV / V^T into group 0's loop
                        if g == 0:
                            if nt == 1:
                                emit_k(1, 0, nc.vector)
                            elif nt == 2:
                                emit_k(1, 1, nc.gpsimd)
                            elif nt == 3:
                                emit_k(1, 2, nc.vector)
                            elif nt == 4:
                                emit_k(1, 3, nc.gpsimd)
                            elif nt == 5:
                                emit_q(1, 0, nc.vector)
                            elif nt == 6:
                                emit_q(1, 1, nc.gpsimd)
                            elif nt == 7:
                                nc.scalar.dma_start(out=wp_sb, in_=wp_d.rearrange("(kt p) (ot o) -> p kt ot o", p=128, o=128))
                                nc.scalar.dma_start(out=b_sb, in_=b_d.rearrange("(ot p) one -> p ot one", p=128))
                            if nt < 13:
                                emit_v(nt + 1, nc.gpsimd if nt % 2 else nc.vector)
                        if nt < 14:
                            emit_st_act(nt, 1, ex)
                            emit_st_trick(nt, 1, 2, nc.gpsimd, ex)
                            emit_st_trick(nt, 1, 3, nc.vector, ex)
                        if nt >= 1:
                            emit_av(nt - 1, prev_ex, prev_w, [0, 1])
                        if nt < 14:
                            prev_ex = ex
                            prev_w = NT_SIZES[nt]

                    # ---- normalize O^T (per-partition scale) into bf16 ----
                    def emit_proj(mc):
                        for ot in range(2):
                            yp = ps.tile([128, MC], F32, tag="st2" if ot else "st3", name="yp", bufs=1)
                            for g2 in range(2):
                                nc.tensor.matmul(
                                    yp[:, 0:MC],
                                    lhsT=wp_sb[:, g2, ot, :],
                                    rhs=on_sb[:, g2, ds(mc * MC, MC)],
                                    start=(g2 == 0),
                                    stop=(g2 == 1),
                                )
                            (nc.vector if ot == 0 else nc.gpsimd).tensor_scalar_add(
                                y_sb[:, ot, ds(mc * MC, MC)], yp[:, 0:MC], b_sb[:, ot, :]
                            )
                            # store each quarter as soon as its bias lands
                            (nc.sync if ot == 0 else nc.scalar).dma_start(
                                out=y_d.rearrange("(ot p) m -> ot p m", p=128)[ot, :, ds(mc * MC, MC)],
                                in_=y_sb[:, ot, ds(mc * MC, MC)],
                            )

                    # ---- normalize O^T: one fused broadcast-multiply per
                    # head pair (PSUM -> bf16 SBUF), then transpose ----
                    for j, eng in ((0, nc.vector), (1, nc.gpsimd)):
                        rs14 = sb.tile([128, 7, 2, 1], F32, tag=f"rs{g}{j}", name="rs14")
                        nc.vector.reciprocal(rs14, ot_ps[j][:, :, :, 32:33])
                        out_v = on_t_sb[:, :, ds(128 * g + 64 * j, 64)].rearrange(
                            "p mt (hh c) -> p mt hh c", hh=2
                        )
                        in0, in1 = broadcast_tensor_aps(ot_ps[j][:, :, :, 0:32], rs14)
                        eng.tensor_tensor(out_v, in0, in1, op=ALU.mult)
                    if g == 1:
                        # keep the PE p-state warm through the normalize window
                        for i in range(4):
                            wuT = ps.tile([128, MC], F32, tag="st2" if i % 2 else "st3", name="wut", bufs=1)
                            nc.tensor.matmul(wuT[:, 0:128], lhsT=wu_in, rhs=wu_in, start=True, stop=True)
                    for mt in range(7):
                        mw = MT_SIZES[mt]
                        if g == 0:
                            # O^T -> O via the DMA xbar transpose (HWDGE is
                            # idle mid-kernel; sync queue only — the scalar
                            # queue would block ACT's in-order SEQ)
                            nc.sync.dma_start_transpose(
                                on_sb[:, g, ds(mt * 128, mw)],
                                on_t_sb[:mw, mt, ds(g * 128, 128)],
                            )
                        else:
                            # tail: PE transpose (bf16) + spread PSUM->SBUF copies
                            tp = ps.tile([128, 128], BF16, tag="st2" if mt % 2 else "st3", name="tp", bufs=1)
                            nc.tensor.transpose(
                                tp[:, :mw],
                                on_t_sb[:mw, mt, ds(g * 128, 128)],
                                ident[:mw, :mw],
                            )
                            ceng = (nc.vector, nc.gpsimd, nc.scalar)[mt % 3]
                            if ceng is nc.scalar:
                                nc.scalar.activation(
                                    on_sb[:, g, ds(mt * 128, mw)], tp[:, :mw], AF.Copy
                                )
                            else:
                                ceng.tensor_copy(on_sb[:, g, ds(mt * 128, mw)], tp[:, :mw])
                            # projection chunks as soon as their columns exist
                            if mt == 3:
                                emit_proj(0)
                            if mt == 6:
                                emit_proj(1)



    nc.compile()
    return nc


_NC = None


def _get_nc():
    global _NC
    if _NC is None:
        _NC = build_program()
    return _NC


def make_in_maps(x, w_qkv, w_proj, b_proj):
    import ml_dtypes

    x = np.asarray(x, np.float32)
    w_qkv = np.asarray(w_qkv, np.float32)
    w_proj = np.asarray(w_proj, np.float32)
    b_proj = np.asarray(b_proj, np.float32)
    P = x.shape[0]
    xf = np.ascontiguousarray(x.reshape(P, C, N))
    wqT = np.ascontiguousarray((w_qkv[0:C] * SCALE).T)
    wkT = np.ascontiguousarray(w_qkv[C : 2 * C].T)
    wvT = np.ascontiguousarray(w_qkv[2 * C : 3 * C].T)
    wpT = np.ascontiguousarray(w_proj.T.astype(ml_dtypes.bfloat16))
    bias = np.ascontiguousarray(b_proj.reshape(C, 1))
    in_maps = []
    for core in range(N_CORES):
        p, mh = divmod(core, 2)
        if mh == 0:
            xp = xf[p]
        else:
            # rotate the n axis so this core's query half comes first
            xp = np.concatenate([xf[p][:, M:], xf[p][:, :M]], axis=1)
        in_maps.append(
            {
                "xf": np.ascontiguousarray(xp),
                "wqT": wqT,
                "wkT": wkT,
                "wvT": wvT,
                "wpT": wpT,
                "bias": bias,
            }
        )
    return in_maps


def assemble_output(per_core_y, x_shape):
    P, B, _, H, W, D = x_shape
    y = np.empty((P, C, N), np.float32)
    for core in range(N_CORES):
        p, mh = divmod(core, 2)
        y[p][:, mh * M : (mh + 1) * M] = per_core_y[core]
    return y.reshape(P, B, C, H, W, D)


def kernel(x, w_qkv, w_proj, b_proj):
    nc = _get_nc()
    in_maps = make_in_maps(x, w_qkv, w_proj, b_proj)
    res = run_bass_kernel_spmd(nc, in_maps, core_ids=list(range(N_CORES)))
    return assemble_output([res.results[c]["y"] for c in range(N_CORES)], x.shape)
